# revision 15
# baseline (speedup 1.0000x reference)
"""CORDIV stochastic-computing division kernel for Trainium2 (8 NeuronCores).

Every stream in this problem is a bitstream ({0,1}), so the host packs 32
lanes into one uint32 word and the device runs the whole recurrence with
bitwise ops:

    q[t] = dvs[t] ? dvd[t] : hq          (per lane)
         = (hq AND a[t]) OR b[t]         (bitwise, 32 lanes/word, exact)
    a[t] = ~dvs[t]  (pass-through mask), b[t] = dvd[t] AND dvs[t]

where hq = q[t-1-r_t] (or a packed sr_init row for t-1-r_t < 0); the tiny
rng_table gather schedule is resolved on the host, so the device kernel is
a static DAG of bitwise tensor_tensor ops on [128, n*W] u32 tiles
(W = N/(cores*128*32) words/partition/step).

Structure (all of it measured on this stack, see docstrings below):
  * Steps live in SBUF in dependency-level order, each level's steps
    ordered by their source's position in the previous level. All sources
    of one level are then contiguous, so the ANDs of a level merge into
    1-2 wide DVE ops and the ORs into exactly one — 13 DVE instructions
    total instead of 32 (per-instruction overhead dominates at this tile
    size). sr_init rows are duplicated host-side so level 1 is one op too,
    and ride in the same DRAM tensor as the selector planes.
  * 2 loads + 2 stores per body, split across the two HWDGE queues
    (SP/ACT) with ~equal bytes. gpsimd/SWDGE is avoided entirely (breaks
    under For_i on this runtime).
  * Tile's For_i inserts a full all-engine barrier + DMA-sem rewind every
    iteration (no cross-iteration overlap), so for REPS>1 the body is
    software-pipelined INNER times inside one iteration with 2 bodies of
    load lookahead: loads never sit behind stores in queue program order.

HBM traffic per core per rep is ~1.56 MiB (1.07 MiB packed selector
planes + sr in, 0.5 MiB packed quotient out) vs ~8 MiB for the u8
baseline; measured steady-state ~6.6 us/rep across 8 cores ≈ the shared
HBM roofline (~290 GB/s/core), down from the 51 us baseline.

Sharding: lane dimension N split evenly across 8 cores (data parallel,
no communication).
"""

import numpy as np

import concourse.bass as bass
import concourse.mybir as mybir
from concourse.tile import TileContext
from concourse.bass_utils import run_bass_kernel_spmd

N_CORES = 8
P = 128  # SBUF partitions
LW = 32  # lanes per u32 word

_nc_cache: dict = {}
LAST_RESULTS = None  # test harness introspection
REPS = 1  # >1: HW-loop reps (timing harness only; output unchanged)
INNER = 16  # software-pipelined bodies per For_i iteration when REPS>1
LOOKAHEAD = 3  # bodies of load lookahead in the software pipeline
COARSE = True  # 2 loads + 2 stores per body (vs per-level chunks)


def _schedule(T, buf_dep, rng_table):
    """Host-side resolution of the shift-register gather into a static DAG.

    Returns (sched, sr_rows): sched[t] = ("q", j) meaning src is quotient row
    j, or ("s", k) meaning src is the k-th entry of sr_rows (a compacted list
    of the sr_init rows actually referenced).
    """
    rng = [int(rng_table[t % buf_dep]) for t in range(T)]
    sched = []
    for t in range(T):
        r = rng[t]
        j = t - 1 - r
        if j >= 0:
            sched.append(("q", j))
        else:
            sched.append(("s", r - t))
    sr_rows = sorted({k for kind, k in sched if kind == "s"})
    row_pos = {k: i for i, k in enumerate(sr_rows)}
    sched = [(kind, k if kind == "q" else row_pos[k]) for kind, k in sched]
    return tuple(sched), sr_rows


def _layout(sched):
    """Level-ordered step layout + merged-op plan.

    Returns (order, pos, levels, and_groups): `order` is the step id at
    each layout position (level-major; within a level sorted by the
    source's layout position so consecutive steps have consecutive
    sources); and_groups is a list of (steps_run, src_kind,
    src_first_pos), each run being one DVE AND op. Level-1 runs
    reference duplicated sr rows (kind "s") shipped in run order.
    """
    T = len(sched)
    depth = [0] * T
    for t in range(T):
        kind, j = sched[t]
        depth[t] = 1 if kind == "s" else depth[j] + 1
    levels = []
    for d in range(1, max(depth) + 1):
        levels.append([t for t in range(T) if depth[t] == d])

    order = []
    pos = {}
    and_groups = []
    for li, lv in enumerate(levels):
        if li == 0:
            lv_sorted = sorted(lv)
        else:
            lv_sorted = sorted(lv, key=lambda t: (pos[sched[t][1]], t))
        for t in lv_sorted:
            pos[t] = len(order)
            order.append(t)
        if li == 0:
            and_groups.append((lv_sorted, "s", 0))
        else:
            run = [lv_sorted[0]]
            for t in lv_sorted[1:]:
                if pos[sched[t][1]] == pos[sched[run[-1]][1]] + 1:
                    run.append(t)
                else:
                    and_groups.append((run, "q", pos[sched[run[0]][1]]))
                    run = [t]
            and_groups.append((run, "q", pos[sched[run[0]][1]]))
        levels[li] = lv_sorted
    return order, pos, levels, and_groups


def _legalize_waits(nc):
    """Make the emitted BIR digestible by this walrus build.

    codegen accepts at most ONE sync wait per instruction (any opcode,
    Drain included). Extra waits are hoisted onto preceding same-engine
    NoOps — engines execute their streams in order, so blocking
    semantics are identical. (InstIncSwdgeSem rewriting kept for safety
    but unused: no SWDGE queues here.)
    """
    n = 0
    for blk in nc.m.functions[0].blocks:
        new_insts = []
        for inst in blk.instructions:
            if type(inst).__name__ == "InstIncSwdgeSem":
                if inst._mode == "add":
                    continue
                assert inst._mode == "sub", inst._mode
                for i, (val, name) in enumerate(
                    zip(inst._sem_values, inst._sem_names)
                ):
                    if val == 0:
                        continue
                    upd = mybir.SyncUpdate(
                        sync_type="semaphore",
                        id=inst._sem_id_base + i,
                        update_mode="sem-sub-imm",
                        update_value=val,
                        ant_name=name,
                    )
                    new_insts.append(
                        mybir.InstNoOp(
                            name=f"{inst.name}_swdgesem_{n}",
                            engine=inst.engine,
                            ins=[],
                            outs=[],
                            sync_info=mybir.SyncInfo(
                                on_wait=[], on_update=[upd]
                            ),
                        )
                    )
                    n += 1
            else:
                new_insts.append(inst)
        blk.instructions = new_insts
    for blk in nc.m.functions[0].blocks:
        new_insts = []
        for inst in blk.instructions:
            si = inst.sync_info
            waits = list(si.on_wait) if si is not None and si.on_wait is not None else []
            if len(waits) > 1 and inst.opcode != "ISA":
                for w in waits[:-1]:
                    nop = mybir.InstNoOp(
                        name=f"{inst.name}_waitnop_{n}",
                        engine=inst.engine,
                        ins=[],
                        outs=[],
                        sync_info=mybir.SyncInfo(on_wait=[w], on_update=[]),
                    )
                    new_insts.append(nop)
                    n += 1
                inst.sync_info = mybir.SyncInfo(
                    on_wait=[waits[-1]], on_update=list(si.on_update or [])
                )
            new_insts.append(inst)
        blk.instructions = new_insts
    return nc


def _build(
    T,
    NS,
    sched,
    reps=1,
    inner=None,
    legalize=True,
    lookahead=None,
    coarse=None,
    sr_const=None,
):
    """Emit the per-core Bass/Tile module. NS = lanes per core.

    sr_const: optional tuple of 0/1 per level-1 step (layout order) when
    every referenced sr_init row is lane-constant (the i%2 init pattern).
    Then hq is a known constant for level 1: sr=0 slots are q = b (copy),
    sr=1 slots are q = a | b — no AND, no sr block, and no a-plane rows
    for the sr=0 slots, saving ~8% of the DMA bytes.
    """
    if inner is None:
        inner = INNER
    if lookahead is None:
        lookahead = LOOKAHEAD
    if coarse is None:
        coarse = COARSE
    # the For_i iteration barrier precludes cross-iteration overlap, so
    # multi-buffering would only waste SBUF in the looped timing builds
    bufs = 2 if reps == 1 else 1
    NSW = NS // LW  # u32 words per step per core
    W = NSW // P  # words per partition per step
    assert NSW % P == 0
    u32 = mybir.dt.uint32
    order, pos, levels, and_groups = _layout(sched)
    n_sr = len(levels[0])  # duplicated sr rows, one per level-1 step
    n_lv = len(levels)

    if sr_const is not None:
        assert len(sr_const) == len(levels[0])
        n_sr = 0
        # level-1 a-rows are only needed where hq is constant-1
        l1_a = [t for i, t in enumerate(levels[0]) if sr_const[i]]
    else:
        l1_a = levels[0]
    nrow = n_sr + len(l1_a) + len(levels[0]) + 2 * sum(
        len(lv) for lv in levels[1:]
    )

    nc = bass.Bass()
    # single input tensor: [sr rows | per level: a-block, b-block]
    bits = nc.dram_tensor("bits", [P, nrow * W], u32, kind="ExternalInput")
    out = nc.dram_tensor("quotient", [P, T * W], u32, kind="ExternalOutput")

    AND = mybir.AluOpType.bitwise_and
    OR = mybir.AluOpType.bitwise_or

    # bits offsets: sr block first, then per level a-block/b-block
    a_off = {}
    b_off = {}
    off = n_sr * W
    lvl_bound = [0]  # chunk boundaries INCLUDE the sr block in chunk 0
    for li, lv in enumerate(levels):
        a_rows = l1_a if li == 0 else lv
        for i, t in enumerate(a_rows):
            a_off[t] = off + i * W
        for i, t in enumerate(lv):
            b_off[t] = off + (len(a_rows) + i) * W
        off += (len(a_rows) + len(lv)) * W
        lvl_bound.append(off)
    assert off == nrow * W

    # queue plans: (engine_name, [level_indices]) for bits loads (chunk 0
    # includes the sr block). Stores are (engine, pos_range), issued after
    # all covering levels complete. Contiguous level runs coalesce into a
    # single DMA, so coarse mode is 2 loads + 2 stores per body.
    lv_start = [sum(len(x) for x in levels[:i]) for i in range(n_lv + 1)]
    if n_lv >= 5:
        if coarse and sr_const is not None:
            # word-granular split inside L3's b-block balances the queues
            # at 6.0 / 5.75 KiB/partition (loads+stores)
            split = lvl_bound[2] + (len(levels[2]) + 3) * W
            load_plan = [("sp", (0, split)), ("act", (split, nrow * W))]
            store_plan = [
                ("sp", (lv_start[3], lv_start[5])),
                ("act", (lv_start[0], lv_start[3])),
            ]
        elif coarse:
            load_plan = [
                ("sp", (lvl_bound[0], lvl_bound[2])),
                ("act", (lvl_bound[2], lvl_bound[5])),
            ]
            store_plan = [
                ("sp", (lv_start[0], lv_start[3])),
                ("act", (lv_start[3], lv_start[5])),
            ]
        else:
            load_plan = [
                ("sp", (lvl_bound[i], lvl_bound[i + 1]))
                if e == "sp"
                else ("act", (lvl_bound[i], lvl_bound[i + 1]))
                for i, e in enumerate(["sp", "act", "sp", "act", "sp"])
            ]
            store_plan = [
                ("act", (lv_start[0], lv_start[2])),
                ("sp", (lv_start[2], lv_start[4])),
                ("act", (lv_start[4], lv_start[5])),
            ]
    else:
        load_plan = [("sp", (0, lvl_bound[n_lv]))]
        store_plan = [("act", (0, lv_start[n_lv]))]

    with TileContext(nc) as tc:
        with (
            tc.tile_pool(name="in", bufs=bufs) as pin,
            tc.tile_pool(name="q", bufs=bufs) as pq,
        ):
            eng = {"sp": nc.sync, "act": nc.scalar}

            def load_part(tag):
                bt = pin.tile([P, nrow * W], u32, tag=f"bits{tag}")
                for e, (c0, c1) in load_plan:
                    eng[e].dma_start(bt[:, c0:c1], bits[:, c0:c1])
                return bt

            def compute_part(tag, bt):
                qt = pq.tile([P, T * W], u32, tag=f"q{tag}")
                for li, lv in enumerate(levels):
                    if li == 0 and sr_const is not None:
                        # constant hq: runs of sr=0 slots are q = b (copy),
                        # runs of sr=1 slots are q = a | b
                        i = 0
                        while i < len(lv):
                            j = i
                            while (
                                j + 1 < len(lv)
                                and sr_const[j + 1] == sr_const[i]
                            ):
                                j += 1
                            n = j - i + 1
                            p0 = pos[lv[i]]
                            dst = qt[:, p0 * W : (p0 + n) * W]
                            b0 = b_off[lv[i]]
                            bm = bt[:, b0 : b0 + n * W]
                            if sr_const[i]:
                                a0 = a_off[lv[i]]
                                am = bt[:, a0 : a0 + n * W]
                                nc.vector.tensor_tensor(dst, am, bm, OR)
                            else:
                                nc.vector.tensor_copy(dst, bm)
                            i = j + 1
                        continue
                    lv_set = set(lv)
                    for run, kind, src0 in [
                        g for g in and_groups if g[0][0] in lv_set
                    ]:
                        n = len(run)
                        p0 = pos[run[0]]
                        dst = qt[:, p0 * W : (p0 + n) * W]
                        if kind == "s":
                            # sr rows are the first n_sr*W words of bt
                            src = bt[:, 0 : n * W]
                        else:
                            src = qt[:, src0 * W : (src0 + n) * W]
                        am = bt[:, a_off[run[0]] : a_off[run[0]] + n * W]
                        nc.vector.tensor_tensor(dst, src, am, AND)
                    p0 = lv_start[li]
                    n = len(lv)
                    dst = qt[:, p0 * W : (p0 + n) * W]
                    bm = bt[:, b_off[lv[0]] : b_off[lv[0]] + n * W]
                    nc.vector.tensor_tensor(dst, dst, bm, OR)
                return qt

            def store_part(tag, qt):
                for e, (s0, s1) in store_plan:
                    eng[e].dma_start(
                        out[:, s0 * W : s1 * W], qt[:, s0 * W : s1 * W]
                    )

            def pipelined(n_bodies, tagp):
                # software pipeline: body k+lookahead's loads are emitted
                # (and so sit in each DMA queue) before body k's stores
                la = min(lookahead, n_bodies)
                loaded = [load_part(f"{tagp}{i}") for i in range(la)]
                for k in range(n_bodies):
                    if k + la < n_bodies:
                        loaded.append(load_part(f"{tagp}{k + la}"))
                    qt = compute_part(f"{tagp}{k}", loaded[k])
                    store_part(f"{tagp}{k}", qt)

            if reps == 1:
                bt = load_part("0")
                qt = compute_part("0", bt)
                store_part("0", qt)
            else:
                trips, rem = divmod(reps, inner)
                if trips > 0:
                    with tc.For_i(0, trips, 1):
                        pipelined(inner, "L")
                if rem:
                    # reuse the loop bodies' tags: no extra SBUF footprint
                    pipelined(rem, "L")
    return _legalize_waits(nc) if legalize else nc


def kernel(dividend, divisor, sr_init, rng_table):
    global LAST_RESULTS
    rng_host = np.asarray(rng_table).astype(np.int64)

    dividend = np.asarray(dividend)
    divisor = np.asarray(divisor)
    T, N = dividend.shape
    buf_dep = np.asarray(sr_init).shape[0]
    assert N % (N_CORES * P * LW) == 0, N
    NS = N // N_CORES
    NSW = NS // LW
    W = NSW // P

    sched, sr_rows = _schedule(T, buf_dep, rng_host)
    order, pos, levels, and_groups = _layout(sched)
    # runtime specialization: if every referenced sr_init row is
    # lane-constant (the i%2 init pattern), level 1 needs no sr data and
    # no a-planes for the constant-0 slots
    sr_np = np.asarray(sr_init)
    l1_rows = [sr_np[sr_rows[sched[t][1]]] for t in levels[0]]
    if all(float(r.min()) == float(r.max()) for r in l1_rows):
        sr_const = tuple(int(r.flat[0]) for r in l1_rows)
    else:
        sr_const = None

    key = (T, NS, sched, REPS, INNER, LOOKAHEAD, COARSE, sr_const)
    nc = _nc_cache.get(key)
    if nc is None:
        nc = _build(T, NS, sched, reps=REPS, sr_const=sr_const)
        _nc_cache[key] = nc

    # bitstreams -> packed bitplanes: a = pass-through mask (~dvs),
    # b = emitted value (dvd & dvs); q = (hq & a) | b  exactly
    dvs = divisor.astype(np.uint8)
    dvd = dividend.astype(np.uint8)
    a32 = np.packbits(1 - dvs, axis=1).view(np.uint32)  # [T, N/32]
    b32 = np.packbits(dvd & dvs, axis=1).view(np.uint32)

    # [sr rows (general path only) | per level: a-block rows then b-block]
    plane_rows = []
    for li, lv in enumerate(levels):
        if li == 0 and sr_const is not None:
            a_rows = [t for i, t in enumerate(lv) if sr_const[i]]
        else:
            a_rows = lv
        plane_rows += [("a", t) for t in a_rows] + [("b", t) for t in lv]
    blocks = [np.stack([(a32 if p == "a" else b32)[t] for p, t in plane_rows])]
    if sr_const is None:
        sr_sel = np.stack(l1_rows).astype(np.uint8)
        blocks.insert(0, np.packbits(sr_sel, axis=1).view(np.uint32))
    stacked = np.concatenate(blocks)

    in_maps = []
    nrow = stacked.shape[0]
    for c in range(N_CORES):
        sl = slice(c * NSW, (c + 1) * NSW)
        bits_c = np.ascontiguousarray(
            stacked[:, sl].reshape(nrow, P, W).transpose(1, 0, 2)
        ).reshape(P, nrow * W)
        in_maps.append({"bits": bits_c})

    res = run_bass_kernel_spmd(nc, in_maps, core_ids=list(range(N_CORES)))
    LAST_RESULTS = res
    qw = np.concatenate(
        [
            m["quotient"].reshape(P, T, W).transpose(1, 0, 2).reshape(T, NSW)
            for m in res.results
        ],
        axis=1,
    )  # [T(layout order), N/32] u32
    inv = np.array([pos[t] for t in range(T)])
    qw = qw[inv]  # back to step order
    q = np.unpackbits(np.ascontiguousarray(qw).view(np.uint8), axis=1)
    return q.astype(np.float32)
